# revision 25
# baseline (speedup 1.0000x reference)
"""Multi-head attention (B=4, S=2048, D=1024, H=16) on 8 trn2 NeuronCores.

Sharding: batch x head-group (Megatron TP). Core c handles batch c//2 and
head-group c%2 (8 heads = 512 dims). Each core:
  - projects V first, then per head-pair projects Q,K transposed ([dg, S])
    interleaved with that pair's attention (overlaps PE-heavy projection
    with ACT-heavy softmax),
  - computes causal attention per head in transposed layout
    (S^T = K @ Q^T so the P@V matmul needs no transpose; softmax row-sums
    come from a ones-column appended to V; no max-subtraction is needed
    because scores are O(1) and masked lanes underflow exp to exactly 0),
  - diagonal chunks compute only the causal column range (lo = r*128),
  - computes the partial out-projection over its 512 dims.
The host sums the two partials per batch (row-parallel unshard).
"""

import numpy as np

import concourse.bass as bass
import concourse.tile as tile
import concourse.mybir as mybir
from concourse import bacc
from concourse import bass_utils
from concourse.bass_interp import get_hw_module

dt = mybir.dt

B, S, D, H, HD = 4, 2048, 1024, 16, 64
NCORES = 8
DG = 512          # dims per head-group (8 heads)
NEG = -10000.0
F32, F16 = dt.float32, dt.float16

LAST_RESULTS = None  # stash for test.py (exec_time_ns etc.)
_PROG_CACHE = {}


def _build(causal: bool, reps: int = 1):
    key = (causal, reps)
    if key in _PROG_CACHE:
        return _PROG_CACHE[key]
    nc = bacc.Bacc(
        "TRN2",
        target_bir_lowering=False,
        debug=False,
        enable_asserts=True,
        num_devices=NCORES,
    )

    qT = nc.dram_tensor("qT", [D, S], F16, kind="ExternalInput").ap()
    kT = nc.dram_tensor("kT", [D, S], F16, kind="ExternalInput").ap()
    vT = nc.dram_tensor("vT", [D, S], F16, kind="ExternalInput").ap()
    wq = nc.dram_tensor("wq", [D, DG], F16, kind="ExternalInput").ap()
    wk = nc.dram_tensor("wk", [D, DG], F16, kind="ExternalInput").ap()
    wv = nc.dram_tensor("wv", [D, DG], F16, kind="ExternalInput").ap()
    wo = nc.dram_tensor("wo", [DG, D], F16, kind="ExternalInput").ap()
    bq = nc.dram_tensor("bq", [4, 128], F32, kind="ExternalInput").ap()
    bk = nc.dram_tensor("bk", [4, 128], F32, kind="ExternalInput").ap()
    bv = nc.dram_tensor("bv", [1, DG], F32, kind="ExternalInput").ap()
    bo = nc.dram_tensor("bo", [1, D], F32, kind="ExternalInput").ap()
    cm = nc.dram_tensor("cm", [128, 128], F16, kind="ExternalInput").ap()
    ident = nc.dram_tensor("ident", [128, 128], F16, kind="ExternalInput").ap()
    am = None
    if not causal:
        am = nc.dram_tensor("am", [S, S], F32, kind="ExternalInput").ap()
    y = nc.dram_tensor("y", [S, D], F32, kind="ExternalOutput").ap()

    from contextlib import nullcontext

    with tile.TileContext(nc) as tc:
        loop_ctx = tc.For_i(0, reps, 1) if reps > 1 else nullcontext()
        with loop_ctx:
         with (
            tc.tile_pool(name="xload", bufs=24) as xload,
            tc.tile_pool(name="wpool", bufs=8) as wpool,
            tc.tile_pool(name="wo16", bufs=4) as wo16p,
            tc.tile_pool(name="consts", bufs=1) as consts,
            tc.tile_pool(name="qtp", bufs=2) as qtp,
            tc.tile_pool(name="ktp", bufs=2) as ktp,
            tc.tile_pool(name="vop", bufs=16) as vop,
            tc.tile_pool(name="atp", bufs=16) as atp,
            tc.tile_pool(name="ppool", bufs=3) as ppool,
            tc.tile_pool(name="small", bufs=2) as small,
            tc.tile_pool(name="ystage", bufs=1) as ystage,
            tc.tile_pool(name="ampool", bufs=2) as ampool,
            tc.tile_pool(name="psbig", bufs=3, space="PSUM") as psbig,
            tc.tile_pool(name="pssmall", bufs=2, space="PSUM") as pssmall,
        ):
            # ---- constants -------------------------------------------------
            ones16 = consts.tile([1, 128], F16, tag="ones16", name="ones16")
            nc.vector.memset(ones16[:], 1.0)

            # causal triangle (f16, added onto scores via identity-matmul)
            cmt = consts.tile([128, 128], F16, tag="cmt", name="cmt")
            nc.sync.dma_start(cmt[:], cm[:])
            idt = consts.tile([128, 128], F16, tag="idt", name="idt")
            nc.sync.dma_start(idt[:], ident[:])

            bqt = [consts.tile([128, 1], F32, tag=f"bqt{i}", name=f"bqt{i}") for i in range(4)]
            bkt = [consts.tile([128, 1], F32, tag=f"bkt{i}", name=f"bkt{i}") for i in range(4)]
            for i in range(4):
                nc.sync.dma_start(bqt[i][:], bq[i : i + 1, :].transpose([1, 0]))
                nc.sync.dma_start(bkt[i][:], bk[i : i + 1, :].transpose([1, 0]))
            bvb = consts.tile([128, DG], F32, tag="bvb", name="bvb")
            nc.sync.dma_start(bvb[0:1, :], bv[:])
            nc.gpsimd.partition_broadcast(bvb[:], bvb[0:1, :])
            bob = consts.tile([128, D], F32, tag="bob", name="bob")
            nc.sync.dma_start(bob[0:1, :], bo[:])
            nc.gpsimd.partition_broadcast(bob[:], bob[0:1, :])

            # ---- phase 1: V projection (natural [S, dv] + ones cols) -------
            # V[s, dv] = sum_d x[s, d] * Wg[dv, d]; lhsT = vT[d, s] blocks,
            # rhs = wv[d, dv].  vones tiles: [128, 8*65], col 64 of each head
            # stripe is 1.0 (for softmax row sums).
            vones = [vop.tile([128, 8 * 65], F16, tag="vo", name=f"vo{i}") for i in range(16)]
            xv_tiles = [xload.tile([128, S], F16, tag="x", name=f"xv{i}") for i in range(8)]
            for db in range(8):
                nc.sync.dma_start(xv_tiles[db][:], vT[db * 128 : (db + 1) * 128, :])
            wv_tiles = [wpool.tile([128, DG], F16, tag="wv", name=f"wv{i}") for i in range(8)]
            for db in range(8):
                nc.sync.dma_start(wv_tiles[db][:], wv[db * 128 : (db + 1) * 128, :])

            def emit_vproj(sb):
                ps = pssmall.tile([128, 512], F32, tag="ps")
                for db in range(8):
                    nc.tensor.matmul(
                        ps[:],
                        xv_tiles[db][:, sb * 128 : (sb + 1) * 128],
                        wv_tiles[db][:],
                        start=(db == 0),
                        stop=(db == 7),
                    )
                vt = vones[sb]
                vview = vt[:].rearrange("p (h e) -> p h e", e=65)
                nc.vector.tensor_add(
                    vview[:, :, 0:64],
                    ps[:].rearrange("p (h e) -> p h e", e=64),
                    bvb[:].rearrange("p (h e) -> p h e", e=64),
                )
                nc.vector.memset(vview[:, :, 64:65], 1.0)

            # ---- phase 2: per head-pair QK projection + attention ----------
            # Background PE work (remaining V s-blocks, next pair's QK
            # projection, out-projection chunks) is interleaved into the
            # attention emission stream so the PE never sits behind an
            # exp-wait with ready work elsewhere.
            xq_tiles = [xload.tile([128, S], F16, tag="x", name=f"xq{i}") for i in range(8)]
            for db in range(8):
                nc.sync.dma_start(xq_tiles[db][:], qT[db * 128 : (db + 1) * 128, :])
            xk_tiles = [xload.tile([128, S], F16, tag="x", name=f"xk{i}") for i in range(8)]
            for db in range(8):
                nc.sync.dma_start(xk_tiles[db][:], kT[db * 128 : (db + 1) * 128, :])
            wq_tiles = [wpool.tile([128, DG], F16, tag="wq", name=f"wq{i}") for i in range(8)]
            wk_tiles = [wpool.tile([128, DG], F16, tag="wk", name=f"wk{i}") for i in range(8)]
            for db in range(8):
                nc.sync.dma_start(wq_tiles[db][:], wq[db * 128 : (db + 1) * 128, :])
                nc.sync.dma_start(wk_tiles[db][:], wk[db * 128 : (db + 1) * 128, :])

            wo_tiles = []
            for pidx in range(4):
                wo16 = wo16p.tile([128, D], F16, tag="wo16", name=f"wo16_{pidx}")
                nc.sync.dma_start(wo16[:], wo[pidx * 128 : (pidx + 1) * 128, :])
                wo_tiles.append(wo16)

            att_tiles = [
                atp.tile([128, 512], F16, tag="at", name=f"at{i}") for i in range(16)
            ]
            qk_tiles = {}

            def emit_proj_group(pr, which, sc):
                qtt, ktt = qk_tiles[pr]
                xt, wt, ot, bt = (
                    (xq_tiles, wq_tiles, qtt, bqt[pr])
                    if which == 0
                    else (xk_tiles, wk_tiles, ktt, bkt[pr])
                )
                ps = pssmall.tile([128, 512], F32, tag="ps")
                for db in range(8):
                    nc.tensor.matmul(
                        ps[:],
                        wt[db][:, pr * 128 : (pr + 1) * 128],
                        xt[db][:, sc * 512 : (sc + 1) * 512],
                        start=(db == 0),
                        stop=(db == 7),
                    )
                nc.vector.tensor_scalar_add(
                    ot[:, sc * 512 : (sc + 1) * 512], ps[:], bt[:]
                )

            def emit_attn_block(pr, qc, j, jmax, o_ps):
                qtt, ktt = qk_tiles[pr]
                r = j - 4 * qc
                lo = r * 128 if (causal and r >= 0) else 0
                sp = psbig.tile([128, 1024], F32, tag="ps")
                spv = sp[:].rearrange("p (r c) -> p r c", r=2)
                diag = causal and 0 <= r
                for hh in range(2):
                    nc.tensor.matmul(
                        sp[:, hh * 512 + lo : (hh + 1) * 512],
                        ktt[hh * 64 : (hh + 1) * 64, j * 128 : (j + 1) * 128],
                        qtt[
                            hh * 64 : (hh + 1) * 64,
                            qc * 512 + lo : (qc + 1) * 512,
                        ],
                        start=True,
                        stop=not diag,
                    )
                if diag:
                    # add the causal triangle on the PE: sp += I.T @ cm
                    for hh in range(2):
                        nc.tensor.matmul(
                            sp[:, hh * 512 + lo : hh * 512 + lo + 128],
                            idt[:],
                            cmt[:],
                            start=False,
                            stop=True,
                        )
                if not causal:
                    amt = ampool.tile([128, 512], F32, tag="am")
                    nc.sync.dma_start(
                        amt[:],
                        am[j * 128 : (j + 1) * 128, qc * 512 : (qc + 1) * 512],
                    )
                    for hh in range(2):
                        nc.vector.tensor_add(
                            sp[:, hh * 512 : (hh + 1) * 512],
                            sp[:, hh * 512 : (hh + 1) * 512],
                            amt[:],
                        )
                pt = ppool.tile([128, 1024], F16, tag="p")
                ptv = pt[:].rearrange("p (r c) -> p r c", r=2)
                nc.scalar.activation(
                    ptv[:, :, lo:512],
                    spv[:, :, lo:512],
                    mybir.ActivationFunctionType.Exp,
                    scale=0.125,
                )
                for hh in range(2):
                    lh = pr * 2 + hh
                    nc.tensor.matmul(
                        o_ps[:, hh * 512 + lo : (hh + 1) * 512],
                        vones[j][:, lh * 65 : (lh + 1) * 65],
                        pt[:, hh * 512 + lo : (hh + 1) * 512],
                        start=(j == 0),
                        stop=(j == jmax),
                    )

            def emit_attn_epilogue(pr, qc, o_ps):
                # fast copies + reciprocals release the PSUM accumulator; the
                # divide works from SBUF off the critical path
                ob = small.tile([64, 1024], F32, tag="ob")
                nc.vector.tensor_copy(ob[:, 0:512], o_ps[0:64, 0:512])
                nc.vector.tensor_copy(ob[:, 512:1024], o_ps[0:64, 512:1024])
                recips = []
                for hh in range(2):
                    recip = small.tile([1, 512], F32, tag="recip")
                    nc.vector.reciprocal(
                        recip[:], o_ps[64:65, hh * 512 : (hh + 1) * 512]
                    )
                    recips.append(recip)
                for hh in range(2):
                    rb = small.tile([64, 512], F32, tag="rb")
                    nc.gpsimd.partition_broadcast(rb[:], recips[hh][:])
                    nc.vector.tensor_mul(
                        att_tiles[pr * 4 + qc][hh * 64 : (hh + 1) * 64, :],
                        ob[0:64, hh * 512 : (hh + 1) * 512],
                        rb[:],
                    )

            def emit_outproj(qcb, qi, oc):
                ps = pssmall.tile([128, 512], F32, tag="ps")
                for pidx in range(4):
                    nc.tensor.matmul(
                        ps[:],
                        att_tiles[pidx * 4 + qcb][:, qi * 128 : (qi + 1) * 128],
                        wo_tiles[pidx][:, oc * 512 : (oc + 1) * 512],
                        start=(pidx == 0),
                        stop=(pidx == 3),
                    )
                yt = ystage.tile([128, 512], F32, tag="yt")
                nc.vector.tensor_add(
                    yt[:], ps[:], bob[:, oc * 512 : (oc + 1) * 512]
                )
                qb = qcb * 4 + qi
                nc.sync.dma_start(
                    y[qb * 128 : (qb + 1) * 128, oc * 512 : (oc + 1) * 512], yt[:]
                )

            # stage 0: first V s-blocks + pair-0 QK projection
            for sb in range(4):
                emit_vproj(sb)
            qk_tiles[0] = (
                qtp.tile([128, S], F16, tag="qt", name="qt0"),
                ktp.tile([128, S], F16, tag="kt", name="kt0"),
            )
            for which in range(2):
                for sc in range(4):
                    emit_proj_group(0, which, sc)

            # merged stream: attention blocks with background PE work
            from collections import deque

            bg = deque()
            for sb in range(4, 16):
                bg.append(("v", sb))
            for pr in range(4):
                if pr + 1 < 4:
                    qk_tiles[pr + 1] = (
                        qtp.tile([128, S], F16, tag="qt", name=f"qt{pr + 1}"),
                        ktp.tile([128, S], F16, tag="kt", name=f"kt{pr + 1}"),
                    )
                    for which in range(2):
                        for sc in range(4):
                            bg.append(("proj", pr + 1, which, sc))
                nblocks = 0
                pace = 1 if pr == 3 else max(1, (40 if causal else 64) // (len(bg) + 1))
                qc_order = [3, 2, 1, 0] if (pr == 3 and causal) else [0, 1, 2, 3]
                for qc in qc_order:
                    jmax = 4 * qc + 3 if causal else 15
                    o_ps = psbig.tile([65, 1024], F32, tag="ps", name="ops")
                    for j in range(jmax + 1):
                        emit_attn_block(pr, qc, j, jmax, o_ps)
                        nblocks += 1
                        if bg and nblocks % pace == 0:
                            item = bg.popleft()
                            if item[0] == "v":
                                emit_vproj(item[1])
                            elif item[0] == "proj":
                                emit_proj_group(*item[1:])
                            else:
                                emit_outproj(*item[1:])
                    emit_attn_epilogue(pr, qc, o_ps)
                    if pr == 3:
                        for qi in range(4):
                            for oc in range(2):
                                bg.append(("out", qc, qi, oc))
            while bg:
                item = bg.popleft()
                if item[0] == "v":
                    emit_vproj(item[1])
                elif item[0] == "proj":
                    emit_proj_group(*item[1:])
                else:
                    emit_outproj(*item[1:])

    nc.compile()
    nc.m = get_hw_module(nc.m)
    _PROG_CACHE[key] = nc
    return nc


def _make_in_maps(q, k, v, Wq, bq, Wk, bk, Wv, bv, Wo, bo, causal, m2):
    # causal triangle (S^T layout: [k_local, u]): valid iff u >= k
    kl = np.arange(128)[:, None]
    ul = np.arange(128)[None, :]
    cm = np.where(ul >= kl, 0.0, NEG).astype(np.float16)
    ident = np.eye(128, dtype=np.float16)

    f16 = np.float16
    WqT, WkT, WvT = Wq.T.astype(f16), Wk.T.astype(f16), Wv.T.astype(f16)
    WoT = Wo.T.astype(f16)
    amask = None
    if not causal:
        # additive mask in S^T layout: [k, q] = transpose of [q, k]
        amask = np.where(m2.T == 0, NEG, 0.0).astype(np.float32)

    in_maps = []
    for c in range(NCORES):
        b, g = c // 2, c % 2
        gs = slice(g * DG, (g + 1) * DG)
        im = {
            "qT": np.ascontiguousarray(q[b].T.astype(f16)),
            "kT": np.ascontiguousarray(k[b].T.astype(f16)),
            "vT": np.ascontiguousarray(v[b].T.astype(f16)),
            "wq": np.ascontiguousarray(WqT[:, gs]),
            "wk": np.ascontiguousarray(WkT[:, gs]),
            "wv": np.ascontiguousarray(WvT[:, gs]),
            "wo": np.ascontiguousarray(WoT[gs, :]),
            "bq": bq[gs].reshape(4, 128).astype(np.float32),
            "bk": bk[gs].reshape(4, 128).astype(np.float32),
            "bv": bv[gs].reshape(1, DG).astype(np.float32),
            "bo": (bo if g == 0 else np.zeros_like(bo)).reshape(1, D).astype(np.float32),
            "cm": cm,
            "ident": ident,
        }
        if not causal:
            im["am"] = amask
        in_maps.append(im)
    return in_maps


_LAST_IN_MAPS = None
_LAST_CAUSAL = None


def _timed_run(nc, in_maps, n=64, warmup=4):
    """Compile nc via the bass_exec fast path, put inputs on device once,
    and return mean seconds per pipelined execution over n calls."""
    import time as _time

    import jax
    from jax.experimental.shard_map import shard_map
    from jax.sharding import Mesh, NamedSharding, PartitionSpec

    from concourse import bass2jax, mybir as mb

    bass2jax.install_neuronx_cc_hook()

    partition_name = (
        nc.partition_id_tensor.name if nc.partition_id_tensor is not None else None
    )
    in_names, out_names, out_avals, zero_outs = [], [], [], []
    for alloc in nc.m.functions[0].allocations:
        if not isinstance(alloc, mb.MemoryLocationSet):
            continue
        name = alloc.memorylocations[0].name
        if alloc.kind == "ExternalInput":
            if name != partition_name:
                in_names.append(name)
        elif alloc.kind == "ExternalOutput":
            out_names.append(name)
            shape = tuple(alloc.tensor_shape)
            dtype = mb.dt.np(alloc.dtype)
            out_avals.append(jax.core.ShapedArray(shape, dtype))
            zero_outs.append(np.zeros(shape, dtype))
    n_params = len(in_names)
    all_in_names = in_names + out_names
    if partition_name is not None:
        all_in_names = all_in_names + [partition_name]

    def _body(*args):
        operands = list(args)
        if partition_name is not None:
            operands.append(bass2jax.partition_id_tensor())
        outs = bass2jax._bass_exec_p.bind(
            *operands,
            out_avals=tuple(out_avals),
            in_names=tuple(all_in_names),
            out_names=tuple(out_names),
            lowering_input_output_aliases=(),
            sim_require_finite=True,
            sim_require_nnan=True,
            nc=nc,
        )
        return tuple(outs)

    devices = jax.devices()[:NCORES]
    mesh = Mesh(np.asarray(devices), ("core",))
    spec = NamedSharding(mesh, PartitionSpec("core"))
    concat_in = [
        jax.device_put(
            np.concatenate([np.asarray(in_maps[c][nm]) for c in range(NCORES)], axis=0),
            spec,
        )
        for nm in in_names
    ]
    concat_zero = [
        jax.device_put(np.zeros((NCORES * z.shape[0], *z.shape[1:]), z.dtype), spec)
        for z in zero_outs
    ]

    def _compile():
        return (
            jax.jit(
                shard_map(
                    _body,
                    mesh=mesh,
                    in_specs=(PartitionSpec("core"),) * (n_params + len(out_names)),
                    out_specs=(PartitionSpec("core"),) * len(out_names),
                    check_rep=False,
                ),
                keep_unused=True,
            )
            .lower(*concat_in, *concat_zero)
            .compile()
        )

    f = bass2jax.fast_dispatch_compile(_compile)
    for _ in range(warmup):
        out = f(*concat_in, *concat_zero)
    jax.block_until_ready(out)
    t0 = _time.perf_counter()
    outs = [f(*concat_in, *concat_zero) for _ in range(n)]
    jax.block_until_ready(outs)
    return (_time.perf_counter() - t0) / n


def benchmark(n=64, warmup=4):
    nc = _PROG_CACHE[(_LAST_CAUSAL, 1)]
    return _timed_run(nc, _LAST_IN_MAPS, n=n, warmup=warmup)


def hw_exec_time_ns(n=64, r_hi=9):
    """Absolute per-run HW time: slope between an R=1 and an R=r_hi program
    (the whole kernel wrapped in a hardware loop), cancelling all per-launch
    overhead."""
    t1 = _timed_run(_build(_LAST_CAUSAL, 1), _LAST_IN_MAPS, n=n)
    t2 = _timed_run(_build(_LAST_CAUSAL, r_hi), _LAST_IN_MAPS, n=n)
    return (t2 - t1) / (r_hi - 1) * 1e9


def kernel(**inputs):
    global LAST_RESULTS, _LAST_IN_MAPS, _LAST_CAUSAL
    q = np.asarray(inputs["q"], dtype=np.float32)
    k = np.asarray(inputs["k"], dtype=np.float32)
    v = np.asarray(inputs["v"], dtype=np.float32)
    mask = np.asarray(inputs["mask"])
    Wq = np.asarray(inputs["Wq"], dtype=np.float32)
    bq = np.asarray(inputs["bq"], dtype=np.float32)
    Wk = np.asarray(inputs["Wk"], dtype=np.float32)
    bk = np.asarray(inputs["bk"], dtype=np.float32)
    Wv = np.asarray(inputs["Wv"], dtype=np.float32)
    bv = np.asarray(inputs["bv"], dtype=np.float32)
    Wo = np.asarray(inputs["Wo"], dtype=np.float32)
    bo = np.asarray(inputs["bo"], dtype=np.float32)

    m2 = mask.reshape(S, S)
    causal = bool(np.array_equal(m2 != 0, np.tril(np.ones((S, S), dtype=bool))))

    nc = _build(causal)
    in_maps = _make_in_maps(q, k, v, Wq, bq, Wk, bk, Wv, bv, Wo, bo, causal, m2)
    _LAST_IN_MAPS, _LAST_CAUSAL = in_maps, causal

    res = bass_utils.run_bass_kernel_spmd(nc, in_maps, core_ids=list(range(NCORES)))
    LAST_RESULTS = res

    out = np.empty((B, S, D), dtype=np.float32)
    for b in range(B):
        np.add(res.results[2 * b]["y"], res.results[2 * b + 1]["y"], out=out[b])
    return out


# revision 28
# speedup vs baseline: 1.2898x; 1.2898x over previous
"""Multi-head attention (B=4, S=2048, D=1024, H=16) on 8 trn2 NeuronCores.

Sharding: batch x head-group (Megatron TP). Core c handles batch c//2 and
head-group c%2 (8 heads = 512 dims). Each core:
  - projects V first, then per head-pair projects Q,K transposed ([dg, S])
    interleaved with that pair's attention (overlaps PE-heavy projection
    with ACT-heavy softmax),
  - computes causal attention per head in transposed layout
    (S^T = K @ Q^T so the P@V matmul needs no transpose; softmax row-sums
    come from a ones-column appended to V; no max-subtraction is needed
    because scores are O(1) and masked lanes underflow exp to exactly 0),
  - diagonal chunks compute only the causal column range (lo = r*128),
  - computes the partial out-projection over its 512 dims.
The host sums the two partials per batch (row-parallel unshard).
"""

import numpy as np

import concourse.bass as bass
import concourse.tile as tile
import concourse.mybir as mybir
from concourse import bacc
from concourse import bass_utils
from concourse.bass_interp import get_hw_module

dt = mybir.dt

B, S, D, H, HD = 4, 2048, 1024, 16, 64
NCORES = 8
DG = 512          # dims per head-group (8 heads)
NEG = -10000.0
F32, F16 = dt.float32, dt.float16

LAST_RESULTS = None  # stash for test.py (exec_time_ns etc.)
_PROG_CACHE = {}


def _build(causal: bool, reps: int = 1):
    key = (causal, reps)
    if key in _PROG_CACHE:
        return _PROG_CACHE[key]
    nc = bacc.Bacc(
        "TRN2",
        target_bir_lowering=False,
        debug=False,
        enable_asserts=True,
        num_devices=NCORES,
    )

    qT = nc.dram_tensor("qT", [D, S], F16, kind="ExternalInput").ap()
    kT = nc.dram_tensor("kT", [D, S], F16, kind="ExternalInput").ap()
    vT = nc.dram_tensor("vT", [D, S], F16, kind="ExternalInput").ap()
    wq = nc.dram_tensor("wq", [D, DG], F16, kind="ExternalInput").ap()
    wk = nc.dram_tensor("wk", [D, DG], F16, kind="ExternalInput").ap()
    wv = nc.dram_tensor("wv", [D, DG], F16, kind="ExternalInput").ap()
    wo = nc.dram_tensor("wo", [DG, D], F16, kind="ExternalInput").ap()
    bq = nc.dram_tensor("bq", [4, 128], F32, kind="ExternalInput").ap()
    bk = nc.dram_tensor("bk", [4, 128], F32, kind="ExternalInput").ap()
    bv = nc.dram_tensor("bv", [1, DG], F32, kind="ExternalInput").ap()
    bo = nc.dram_tensor("bo", [1, D], F32, kind="ExternalInput").ap()
    cm = nc.dram_tensor("cm", [128, 128], F16, kind="ExternalInput").ap()
    ident = nc.dram_tensor("ident", [128, 128], F16, kind="ExternalInput").ap()
    am = None
    if not causal:
        am = nc.dram_tensor("am", [S, S], F16, kind="ExternalInput").ap()
    y = nc.dram_tensor("y", [S, D], F32, kind="ExternalOutput").ap()

    from contextlib import nullcontext

    with tile.TileContext(nc) as tc:
        loop_ctx = tc.For_i(0, reps, 1) if reps > 1 else nullcontext()
        with loop_ctx:
         with (
            tc.tile_pool(name="xload", bufs=24) as xload,
            tc.tile_pool(name="wpool", bufs=8) as wpool,
            tc.tile_pool(name="wo16", bufs=4) as wo16p,
            tc.tile_pool(name="consts", bufs=1) as consts,
            tc.tile_pool(name="qtp", bufs=2) as qtp,
            tc.tile_pool(name="ktp", bufs=2) as ktp,
            tc.tile_pool(name="vop", bufs=16) as vop,
            tc.tile_pool(name="atp", bufs=16) as atp,
            tc.tile_pool(name="ppool", bufs=3) as ppool,
            tc.tile_pool(name="small", bufs=2) as small,
            tc.tile_pool(name="ystage", bufs=1) as ystage,
            tc.tile_pool(name="ampool", bufs=2) as ampool,
            tc.tile_pool(name="psbig", bufs=3, space="PSUM") as psbig,
            tc.tile_pool(name="pssmall", bufs=2, space="PSUM") as pssmall,
        ):
            # ---- constants -------------------------------------------------
            ones16 = consts.tile([1, 128], F16, tag="ones16", name="ones16")
            nc.vector.memset(ones16[:], 1.0)

            # causal triangle (f16, added onto scores via identity-matmul)
            cmt = consts.tile([128, 128], F16, tag="cmt", name="cmt")
            nc.sync.dma_start(cmt[:], cm[:])
            idt = consts.tile([128, 128], F16, tag="idt", name="idt")
            nc.sync.dma_start(idt[:], ident[:])

            bqt = [consts.tile([128, 1], F32, tag=f"bqt{i}", name=f"bqt{i}") for i in range(4)]
            bkt = [consts.tile([128, 1], F32, tag=f"bkt{i}", name=f"bkt{i}") for i in range(4)]
            for i in range(4):
                nc.sync.dma_start(bqt[i][:], bq[i : i + 1, :].transpose([1, 0]))
                nc.sync.dma_start(bkt[i][:], bk[i : i + 1, :].transpose([1, 0]))
            bvb = consts.tile([128, DG], F32, tag="bvb", name="bvb")
            nc.sync.dma_start(bvb[0:1, :], bv[:])
            nc.gpsimd.partition_broadcast(bvb[:], bvb[0:1, :])
            bob = consts.tile([128, D], F32, tag="bob", name="bob")
            nc.sync.dma_start(bob[0:1, :], bo[:])
            nc.gpsimd.partition_broadcast(bob[:], bob[0:1, :])

            # ---- phase 1: V projection (natural [S, dv] + ones cols) -------
            # V[s, dv] = sum_d x[s, d] * Wg[dv, d]; lhsT = vT[d, s] blocks,
            # rhs = wv[d, dv].  vones tiles: [128, 8*65], col 64 of each head
            # stripe is 1.0 (for softmax row sums).
            vones = [vop.tile([128, 8 * 65], F16, tag="vo", name=f"vo{i}") for i in range(16)]
            xv_tiles = [xload.tile([128, S], F16, tag="x", name=f"xv{i}") for i in range(8)]
            for db in range(8):
                nc.sync.dma_start(xv_tiles[db][:], vT[db * 128 : (db + 1) * 128, :])
            wv_tiles = [wpool.tile([128, DG], F16, tag="wv", name=f"wv{i}") for i in range(8)]
            for db in range(8):
                nc.sync.dma_start(wv_tiles[db][:], wv[db * 128 : (db + 1) * 128, :])

            def emit_vproj(sb):
                ps = pssmall.tile([128, 512], F32, tag="ps")
                for db in range(8):
                    nc.tensor.matmul(
                        ps[:],
                        xv_tiles[db][:, sb * 128 : (sb + 1) * 128],
                        wv_tiles[db][:],
                        start=(db == 0),
                        stop=(db == 7),
                    )
                vt = vones[sb]
                vview = vt[:].rearrange("p (h e) -> p h e", e=65)
                nc.vector.tensor_add(
                    vview[:, :, 0:64],
                    ps[:].rearrange("p (h e) -> p h e", e=64),
                    bvb[:].rearrange("p (h e) -> p h e", e=64),
                )
                nc.vector.memset(vview[:, :, 64:65], 1.0)

            # ---- phase 2: per head-pair QK projection + attention ----------
            # Background PE work (remaining V s-blocks, next pair's QK
            # projection, out-projection chunks) is interleaved into the
            # attention emission stream so the PE never sits behind an
            # exp-wait with ready work elsewhere.
            xq_tiles = [xload.tile([128, S], F16, tag="x", name=f"xq{i}") for i in range(8)]
            for db in range(8):
                nc.sync.dma_start(xq_tiles[db][:], qT[db * 128 : (db + 1) * 128, :])
            xk_tiles = [xload.tile([128, S], F16, tag="x", name=f"xk{i}") for i in range(8)]
            for db in range(8):
                nc.sync.dma_start(xk_tiles[db][:], kT[db * 128 : (db + 1) * 128, :])
            wq_tiles = [wpool.tile([128, DG], F16, tag="wq", name=f"wq{i}") for i in range(8)]
            wk_tiles = [wpool.tile([128, DG], F16, tag="wk", name=f"wk{i}") for i in range(8)]
            for db in range(8):
                nc.sync.dma_start(wq_tiles[db][:], wq[db * 128 : (db + 1) * 128, :])
                nc.sync.dma_start(wk_tiles[db][:], wk[db * 128 : (db + 1) * 128, :])

            wo_tiles = []
            for pidx in range(4):
                wo16 = wo16p.tile([128, D], F16, tag="wo16", name=f"wo16_{pidx}")
                nc.sync.dma_start(wo16[:], wo[pidx * 128 : (pidx + 1) * 128, :])
                wo_tiles.append(wo16)

            att_tiles = [
                atp.tile([128, 512], F16, tag="at", name=f"at{i}") for i in range(16)
            ]
            qk_tiles = {}

            def emit_proj_group(pr, which, sc):
                qtt, ktt = qk_tiles[pr]
                xt, wt, ot, bt = (
                    (xq_tiles, wq_tiles, qtt, bqt[pr])
                    if which == 0
                    else (xk_tiles, wk_tiles, ktt, bkt[pr])
                )
                ps = pssmall.tile([128, 512], F32, tag="ps")
                for db in range(8):
                    nc.tensor.matmul(
                        ps[:],
                        wt[db][:, pr * 128 : (pr + 1) * 128],
                        xt[db][:, sc * 512 : (sc + 1) * 512],
                        start=(db == 0),
                        stop=(db == 7),
                    )
                nc.vector.tensor_scalar_add(
                    ot[:, sc * 512 : (sc + 1) * 512], ps[:], bt[:]
                )

            def emit_attn_block(pr, qc, j, jmax, o_ps):
                qtt, ktt = qk_tiles[pr]
                r = j - 4 * qc
                lo = r * 128 if (causal and r >= 0) else 0
                sp = psbig.tile([128, 1024], F32, tag="ps")
                spv = sp[:].rearrange("p (r c) -> p r c", r=2)
                diag = causal and 0 <= r
                for hh in range(2):
                    nc.tensor.matmul(
                        sp[:, hh * 512 + lo : (hh + 1) * 512],
                        ktt[hh * 64 : (hh + 1) * 64, j * 128 : (j + 1) * 128],
                        qtt[
                            hh * 64 : (hh + 1) * 64,
                            qc * 512 + lo : (qc + 1) * 512,
                        ],
                        start=True,
                        stop=not diag,
                    )
                if diag:
                    # add the causal triangle on the PE: sp += I.T @ cm
                    for hh in range(2):
                        nc.tensor.matmul(
                            sp[:, hh * 512 + lo : hh * 512 + lo + 128],
                            idt[:],
                            cmt[:],
                            start=False,
                            stop=True,
                        )
                if not causal:
                    amt = ampool.tile([128, 512], F16, tag="am")
                    nc.sync.dma_start(
                        amt[:],
                        am[j * 128 : (j + 1) * 128, qc * 512 : (qc + 1) * 512],
                    )
                    for hh in range(2):
                        nc.vector.tensor_add(
                            sp[:, hh * 512 : (hh + 1) * 512],
                            sp[:, hh * 512 : (hh + 1) * 512],
                            amt[:],
                        )
                pt = ppool.tile([128, 1024], F16, tag="p")
                ptv = pt[:].rearrange("p (r c) -> p r c", r=2)
                nc.scalar.activation(
                    ptv[:, :, lo:512],
                    spv[:, :, lo:512],
                    mybir.ActivationFunctionType.Exp,
                    scale=0.125,
                )
                for hh in range(2):
                    lh = pr * 2 + hh
                    nc.tensor.matmul(
                        o_ps[:, hh * 512 + lo : (hh + 1) * 512],
                        vones[j][:, lh * 65 : (lh + 1) * 65],
                        pt[:, hh * 512 + lo : (hh + 1) * 512],
                        start=(j == 0),
                        stop=(j == jmax),
                    )

            def emit_attn_epilogue(pr, qc, o_ps):
                # fast copies + reciprocals release the PSUM accumulator; the
                # divide works from SBUF off the critical path
                ob = small.tile([64, 1024], F16, tag="ob")
                nc.vector.tensor_copy(ob[:, 0:512], o_ps[0:64, 0:512])
                nc.vector.tensor_copy(ob[:, 512:1024], o_ps[0:64, 512:1024])
                recips = []
                for hh in range(2):
                    recip = small.tile([1, 512], F32, tag="recip")
                    nc.vector.reciprocal(
                        recip[:], o_ps[64:65, hh * 512 : (hh + 1) * 512]
                    )
                    recips.append(recip)
                for hh in range(2):
                    rb = small.tile([64, 512], F32, tag="rb")
                    nc.gpsimd.partition_broadcast(rb[:], recips[hh][:])
                    nc.vector.tensor_mul(
                        att_tiles[pr * 4 + qc][hh * 64 : (hh + 1) * 64, :],
                        ob[0:64, hh * 512 : (hh + 1) * 512],
                        rb[:],
                    )

            def emit_outproj(qcb, qi, oc):
                ps = pssmall.tile([128, 512], F32, tag="ps")
                for pidx in range(4):
                    nc.tensor.matmul(
                        ps[:],
                        att_tiles[pidx * 4 + qcb][:, qi * 128 : (qi + 1) * 128],
                        wo_tiles[pidx][:, oc * 512 : (oc + 1) * 512],
                        start=(pidx == 0),
                        stop=(pidx == 3),
                    )
                yt = ystage.tile([128, 512], F32, tag="yt")
                nc.vector.tensor_add(
                    yt[:], ps[:], bob[:, oc * 512 : (oc + 1) * 512]
                )
                qb = qcb * 4 + qi
                nc.sync.dma_start(
                    y[qb * 128 : (qb + 1) * 128, oc * 512 : (oc + 1) * 512], yt[:]
                )

            # stage 0: first V s-blocks + pair-0 QK projection
            emitted_v = set()
            for sb in range(4):
                emit_vproj(sb)
                emitted_v.add(sb)
            qk_tiles[0] = (
                qtp.tile([128, S], F16, tag="qt", name="qt0"),
                ktp.tile([128, S], F16, tag="kt", name="kt0"),
            )
            for which in range(2):
                for sc in range(4):
                    emit_proj_group(0, which, sc)

            # merged stream: attention blocks with background PE work
            from collections import deque

            bg = deque()
            for sb in range(4, 16):
                bg.append(("v", sb))
            for pr in range(4):
                if pr + 1 < 4:
                    qk_tiles[pr + 1] = (
                        qtp.tile([128, S], F16, tag="qt", name=f"qt{pr + 1}"),
                        ktp.tile([128, S], F16, tag="kt", name=f"kt{pr + 1}"),
                    )
                    for which in range(2):
                        for sc in range(4):
                            bg.append(("proj", pr + 1, which, sc))
                nblocks = 0
                pace = 1 if pr == 3 else max(1, (40 if causal else 64) // (len(bg) + 1))
                qc_order = [3, 2, 1, 0] if (pr == 3 and causal) else [0, 1, 2, 3]

                def pop_bg():
                    item = bg.popleft()
                    if item[0] == "v":
                        emit_vproj(item[1])
                        emitted_v.add(item[1])
                    elif item[0] == "proj":
                        emit_proj_group(*item[1:])
                    else:
                        emit_outproj(*item[1:])

                for qc in qc_order:
                    jmax = 4 * qc + 3 if causal else 15
                    o_ps = psbig.tile([65, 1024], F32, tag="ps", name="ops")
                    for j in range(jmax + 1):
                        # the attention block reads vones[j]: its V-projection
                        # group must already be emitted (Tile orders only
                        # against past writers)
                        while j not in emitted_v:
                            pop_bg()
                        emit_attn_block(pr, qc, j, jmax, o_ps)
                        nblocks += 1
                        if bg and nblocks % pace == 0:
                            pop_bg()
                    emit_attn_epilogue(pr, qc, o_ps)
                    if pr == 3:
                        for qi in range(4):
                            for oc in range(2):
                                bg.append(("out", qc, qi, oc))
            while bg:
                item = bg.popleft()
                if item[0] == "v":
                    emit_vproj(item[1])
                elif item[0] == "proj":
                    emit_proj_group(*item[1:])
                else:
                    emit_outproj(*item[1:])

    nc.compile()
    nc.m = get_hw_module(nc.m)
    _PROG_CACHE[key] = nc
    return nc


def _make_in_maps(q, k, v, Wq, bq, Wk, bk, Wv, bv, Wo, bo, causal, m2):
    # causal triangle (S^T layout: [k_local, u]): valid iff u >= k
    kl = np.arange(128)[:, None]
    ul = np.arange(128)[None, :]
    cm = np.where(ul >= kl, 0.0, NEG).astype(np.float16)
    ident = np.eye(128, dtype=np.float16)

    f16 = np.float16
    WqT, WkT, WvT = Wq.T.astype(f16), Wk.T.astype(f16), Wv.T.astype(f16)
    WoT = Wo.T.astype(f16)
    amask = None
    if not causal:
        # additive mask in S^T layout: [k, q] = transpose of [q, k]
        amask = np.where(m2.T == 0, NEG, 0.0).astype(np.float16)

    in_maps = []
    for c in range(NCORES):
        b, g = c // 2, c % 2
        gs = slice(g * DG, (g + 1) * DG)
        im = {
            "qT": np.ascontiguousarray(q[b].T.astype(f16)),
            "kT": np.ascontiguousarray(k[b].T.astype(f16)),
            "vT": np.ascontiguousarray(v[b].T.astype(f16)),
            "wq": np.ascontiguousarray(WqT[:, gs]),
            "wk": np.ascontiguousarray(WkT[:, gs]),
            "wv": np.ascontiguousarray(WvT[:, gs]),
            "wo": np.ascontiguousarray(WoT[gs, :]),
            "bq": bq[gs].reshape(4, 128).astype(np.float32),
            "bk": bk[gs].reshape(4, 128).astype(np.float32),
            "bv": bv[gs].reshape(1, DG).astype(np.float32),
            "bo": (bo if g == 0 else np.zeros_like(bo)).reshape(1, D).astype(np.float32),
            "cm": cm,
            "ident": ident,
        }
        if not causal:
            im["am"] = amask
        in_maps.append(im)
    return in_maps


_LAST_IN_MAPS = None
_LAST_CAUSAL = None


def _timed_run(nc, in_maps, n=64, warmup=4):
    """Compile nc via the bass_exec fast path, put inputs on device once,
    and return mean seconds per pipelined execution over n calls."""
    import time as _time

    import jax
    from jax.experimental.shard_map import shard_map
    from jax.sharding import Mesh, NamedSharding, PartitionSpec

    from concourse import bass2jax, mybir as mb

    bass2jax.install_neuronx_cc_hook()

    partition_name = (
        nc.partition_id_tensor.name if nc.partition_id_tensor is not None else None
    )
    in_names, out_names, out_avals, zero_outs = [], [], [], []
    for alloc in nc.m.functions[0].allocations:
        if not isinstance(alloc, mb.MemoryLocationSet):
            continue
        name = alloc.memorylocations[0].name
        if alloc.kind == "ExternalInput":
            if name != partition_name:
                in_names.append(name)
        elif alloc.kind == "ExternalOutput":
            out_names.append(name)
            shape = tuple(alloc.tensor_shape)
            dtype = mb.dt.np(alloc.dtype)
            out_avals.append(jax.core.ShapedArray(shape, dtype))
            zero_outs.append(np.zeros(shape, dtype))
    n_params = len(in_names)
    all_in_names = in_names + out_names
    if partition_name is not None:
        all_in_names = all_in_names + [partition_name]

    def _body(*args):
        operands = list(args)
        if partition_name is not None:
            operands.append(bass2jax.partition_id_tensor())
        outs = bass2jax._bass_exec_p.bind(
            *operands,
            out_avals=tuple(out_avals),
            in_names=tuple(all_in_names),
            out_names=tuple(out_names),
            lowering_input_output_aliases=(),
            sim_require_finite=True,
            sim_require_nnan=True,
            nc=nc,
        )
        return tuple(outs)

    devices = jax.devices()[:NCORES]
    mesh = Mesh(np.asarray(devices), ("core",))
    spec = NamedSharding(mesh, PartitionSpec("core"))
    concat_in = [
        jax.device_put(
            np.concatenate([np.asarray(in_maps[c][nm]) for c in range(NCORES)], axis=0),
            spec,
        )
        for nm in in_names
    ]
    concat_zero = [
        jax.device_put(np.zeros((NCORES * z.shape[0], *z.shape[1:]), z.dtype), spec)
        for z in zero_outs
    ]

    def _compile():
        return (
            jax.jit(
                shard_map(
                    _body,
                    mesh=mesh,
                    in_specs=(PartitionSpec("core"),) * (n_params + len(out_names)),
                    out_specs=(PartitionSpec("core"),) * len(out_names),
                    check_rep=False,
                ),
                keep_unused=True,
            )
            .lower(*concat_in, *concat_zero)
            .compile()
        )

    f = bass2jax.fast_dispatch_compile(_compile)
    for _ in range(warmup):
        out = f(*concat_in, *concat_zero)
    jax.block_until_ready(out)
    t0 = _time.perf_counter()
    outs = [f(*concat_in, *concat_zero) for _ in range(n)]
    jax.block_until_ready(outs)
    return (_time.perf_counter() - t0) / n


def benchmark(n=64, warmup=4):
    nc = _PROG_CACHE[(_LAST_CAUSAL, 1)]
    return _timed_run(nc, _LAST_IN_MAPS, n=n, warmup=warmup)


def hw_exec_time_ns(n=64, r_hi=9):
    """Absolute per-run HW time: slope between an R=1 and an R=r_hi program
    (the whole kernel wrapped in a hardware loop), cancelling all per-launch
    overhead."""
    t1 = _timed_run(_build(_LAST_CAUSAL, 1), _LAST_IN_MAPS, n=n)
    t2 = _timed_run(_build(_LAST_CAUSAL, r_hi), _LAST_IN_MAPS, n=n)
    return (t2 - t1) / (r_hi - 1) * 1e9


def kernel(**inputs):
    global LAST_RESULTS, _LAST_IN_MAPS, _LAST_CAUSAL
    q = np.asarray(inputs["q"], dtype=np.float32)
    k = np.asarray(inputs["k"], dtype=np.float32)
    v = np.asarray(inputs["v"], dtype=np.float32)
    mask = np.asarray(inputs["mask"])
    Wq = np.asarray(inputs["Wq"], dtype=np.float32)
    bq = np.asarray(inputs["bq"], dtype=np.float32)
    Wk = np.asarray(inputs["Wk"], dtype=np.float32)
    bk = np.asarray(inputs["bk"], dtype=np.float32)
    Wv = np.asarray(inputs["Wv"], dtype=np.float32)
    bv = np.asarray(inputs["bv"], dtype=np.float32)
    Wo = np.asarray(inputs["Wo"], dtype=np.float32)
    bo = np.asarray(inputs["bo"], dtype=np.float32)

    m2 = mask.reshape(S, S)
    causal = bool(np.array_equal(m2 != 0, np.tril(np.ones((S, S), dtype=bool))))

    nc = _build(causal)
    in_maps = _make_in_maps(q, k, v, Wq, bq, Wk, bk, Wv, bv, Wo, bo, causal, m2)
    _LAST_IN_MAPS, _LAST_CAUSAL = in_maps, causal

    res = bass_utils.run_bass_kernel_spmd(nc, in_maps, core_ids=list(range(NCORES)))
    LAST_RESULTS = res

    out = np.empty((B, S, D), dtype=np.float32)
    for b in range(B):
        np.add(res.results[2 * b]["y"], res.results[2 * b + 1]["y"], out=out[b])
    return out
